# revision 1
# baseline (speedup 1.0000x reference)
"""Trainium2 Bass kernel for nn_LlamaAttention_31782757990403.

Sparse (full + streaming) Llama attention block with W8A8 fake-quant
projections, distributed over 8 NeuronCores.

Sharding (v0, uniform SPMD — one NEFF, no branches):
  Core c owns kv head c (query heads 4c..4c+3).
    - QKV projection: output-channel shard (768 rows of qkv_w per core).
    - Attention: 4 units = 4 batches of head c. Streaming heads (c >= 4)
      get their sink+recent KV packed into the first 1088 positions and
      the remainder disabled via a per-kpos additive mask (-1e9).
    - Attention outputs are AllGathered (feature-major) across cores.
    - Output projection: row shard (512 out channels of o_w per core);
      host concatenates the 8 column slices.

Numerics:
  - Weight / activation int8 fake-quant reproduced exactly: quantized
    values are integers held in bf16 (exact), matmul accumulates f32.
  - Attention runs in fp16 (K, Q, V, P) with f32 PSUM accumulation and a
    constant -4 shift before exp for fp16 range safety (cancels in the
    softmax ratio). Softmax max-subtraction is skipped (scores are far
    from overflow in f32/exp).
  - attn-out quantization on device: round via f32->int32 convert (RNE,
    matches numpy/jax round).
"""

import numpy as np
import ml_dtypes

import concourse.bass as bass
import concourse.mybir as mybir
import concourse.tile as tile
from concourse import bacc, bass_utils
from concourse.masks import make_identity

dt = mybir.dt
AF = mybir.ActivationFunctionType
ALU = mybir.AluOpType
AX = mybir.AxisListType

NH, NKV, HD, HID = 32, 8, 128, 4096
BSZ, QLEN, PLEN = 4, 16, 4096
TOK = BSZ * QLEN                      # 64
G = NH // NKV                         # 4 query heads per kv head
N_CORES = 8
QKV_ROWS = G * HD + 2 * HD            # 768 rows of qkv_w per core
OW_ROWS = HID // N_CORES              # 512 o_w rows per core
NCH = PLEN // HD                      # 32 past-kv chunks of 128
SCL = float(1.0 / np.sqrt(np.float32(HD)))   # 1/sqrt(128)
SHIFT = -4.0                          # exp stability shift (cancels)
NEG = -1.0e9

_prog_cache = {}


def _build_program():
    nc = bacc.Bacc("TRN2", target_bir_lowering=False, debug=False,
                   num_devices=N_CORES)
    f32, f16, bf16, i32 = dt.float32, dt.float16, dt.bfloat16, dt.int32

    def inp(name, shape, d):
        return nc.dram_tensor(name, shape, d, kind="ExternalInput").ap()

    xqT = inp("xqT", [HID, TOK], bf16)
    xs = inp("xs", [TOK, 1], f32)
    wqT = inp("wqT", [HID, QKV_ROWS], bf16)
    wss = inp("wss", [1, QKV_ROWS], f32)
    cosT = inp("cosT", [TOK, 64], f32)
    sinT = inp("sinT", [TOK, 64], f32)
    kT = inp("kT", [BSZ, HD, PLEN], f16)
    vv = inp("vv", [BSZ, PLEN, HD], f16)
    maskv = inp("maskv", [HD, NCH], f32)
    newmask = inp("newmask", [QLEN, TOK], f32)
    owT = inp("owT", [HID, OW_ROWS], bf16)
    ows = inp("ows", [1, OW_ROWS], f32)
    out_ap = nc.dram_tensor("out_slice", [TOK, OW_ROWS], f32,
                            kind="ExternalOutput").ap()

    with tile.TileContext(nc, num_cores=N_CORES) as tc:
        with (
            tc.tile_pool(name="persist", bufs=1) as P1,
            tc.tile_pool(name="kt", bufs=3) as KTP,
            tc.tile_pool(name="vt", bufs=3) as VTP,
            tc.tile_pool(name="pt", bufs=4) as PTP,
            tc.tile_pool(name="work", bufs=2) as WK,
            tc.tile_pool(name="ps_proj", bufs=2, space="PSUM") as PSP,
            tc.tile_pool(name="ps_sc", bufs=2, space="PSUM") as PSS,
            tc.tile_pool(name="ps_o", bufs=2, space="PSUM") as PSO,
            tc.tile_pool(name="ps_m", bufs=2, space="PSUM") as PSM,
            tc.tile_pool(name="dram", bufs=1, space="DRAM") as DR,
        ):
            # ---------- resident loads ----------
            xqT_sb = P1.tile([HD, HID // HD, TOK], bf16)
            nc.sync.dma_start(out=xqT_sb, in_=xqT.rearrange("(c p) t -> p c t", p=HD))
            wqT_sb = P1.tile([HD, HID // HD, QKV_ROWS], bf16)
            nc.sync.dma_start(out=wqT_sb, in_=wqT.rearrange("(c p) f -> p c f", p=HD))
            owT_sb = P1.tile([HD, HID // HD, OW_ROWS], bf16)
            nc.sync.dma_start(out=owT_sb, in_=owT.rearrange("(c p) f -> p c f", p=HD))
            xs_sb = P1.tile([TOK, 1], f32)
            nc.sync.dma_start(out=xs_sb, in_=xs)
            cos_sb = P1.tile([TOK, 64], f32)
            nc.sync.dma_start(out=cos_sb, in_=cosT)
            sin_sb = P1.tile([TOK, 64], f32)
            nc.sync.dma_start(out=sin_sb, in_=sinT)
            mask_sb = P1.tile([HD, NCH], f32)
            nc.sync.dma_start(out=mask_sb, in_=maskv)
            nmask_sb = P1.tile([QLEN, TOK], f32)
            nc.sync.dma_start(out=nmask_sb, in_=newmask)
            # broadcast rows for per-feature scales
            wss_b = P1.tile([TOK, QKV_ROWS], f32)
            nc.sync.dma_start(out=wss_b, in_=bass.AP(
                tensor=wss.tensor, offset=wss.offset, ap=[[0, TOK]] + wss.ap[1:]))
            ows_b = P1.tile([TOK, OW_ROWS], f32)
            nc.sync.dma_start(out=ows_b, in_=bass.AP(
                tensor=ows.tensor, offset=ows.offset, ap=[[0, TOK]] + ows.ap[1:]))
            shift_sb = P1.tile([QLEN, 1], f32)
            nc.vector.memset(shift_sb, SHIFT)
            ident16 = P1.tile([HD, HD], f16)
            make_identity(nc, ident16)
            ident32 = P1.tile([HD, HD], f32)
            make_identity(nc, ident32)

            # ---------- QKV projection ----------
            qkv_sb = P1.tile([TOK, QKV_ROWS], f32)
            for nb in range(2):
                ncols = QKV_ROWS // 2    # 384
                ps = PSP.tile([TOK, ncols], f32, tag="proj", padded_shape=[TOK, 512])
                for kc in range(HID // HD):
                    nc.tensor.matmul(
                        ps, lhsT=xqT_sb[:, kc, :],
                        rhs=wqT_sb[:, kc, nb * ncols:(nb + 1) * ncols],
                        start=(kc == 0), stop=(kc == HID // HD - 1))
                # dequant: * xs (per token/partition) * ws (per feature)
                nc.scalar.activation(out=qkv_sb[:, nb * ncols:(nb + 1) * ncols],
                                     in_=ps, func=AF.Copy, scale=xs_sb[:, 0:1])
            nc.vector.tensor_mul(out=qkv_sb, in0=qkv_sb, in1=wss_b)

            # ---------- RoPE on q (4 heads) and k ----------
            roped = P1.tile([TOK, (G + 1) * HD], f32)
            for seg in range(G + 1):
                b0 = seg * HD
                x1 = qkv_sb[:, b0:b0 + 64]
                x2 = qkv_sb[:, b0 + 64:b0 + HD]
                t1 = WK.tile([TOK, 64], f32, tag="rope1")
                t2 = WK.tile([TOK, 64], f32, tag="rope2")
                nc.vector.tensor_mul(out=t1, in0=x1, in1=cos_sb)
                nc.vector.tensor_mul(out=t2, in0=x2, in1=sin_sb)
                nc.vector.tensor_sub(out=roped[:, b0:b0 + 64], in0=t1, in1=t2)
                nc.vector.tensor_mul(out=t1, in0=x1, in1=sin_sb)
                nc.vector.tensor_mul(out=t2, in0=x2, in1=cos_sb)
                nc.vector.tensor_add(out=roped[:, b0 + 64:b0 + HD], in0=t1, in1=t2)

            qk16 = P1.tile([TOK, (G + 1) * HD], f16)
            nc.vector.tensor_copy(out=qk16, in_=roped)
            v16f = P1.tile([TOK, HD], f16)
            nc.vector.tensor_copy(out=v16f, in_=qkv_sb[:, (G + 2) * HD - HD:])
            # per-batch v tiles at base partition 0: [16 tok, 4 batch, 129]
            v16 = P1.tile([QLEN, BSZ, HD + 1], f16)
            for b in range(BSZ):
                nc.sync.dma_start(out=v16[:, b, 0:HD],
                                  in_=v16f[b * QLEN:(b + 1) * QLEN, :])
            nc.vector.memset(v16[:, :, HD:HD + 1], 1.0)

            # transpose q heads + k: [64, 128] -> [128, 64]
            qT_sb = P1.tile([HD, G + 1, TOK], f16)
            for seg in range(G + 1):
                pst = PSM.tile([HD, TOK], f16, tag="misc")
                nc.tensor.transpose(pst, qk16[:, seg * HD:(seg + 1) * HD],
                                    ident16[0:TOK, 0:TOK])
                nc.vector.tensor_copy(out=qT_sb[:, seg, :], in_=pst)

            # ---------- attention units (4 batches of this core's kv head) ----
            contrib = DR.tile([G * HD, TOK], f32)
            gathered = DR.tile([HID, TOK], f32, addr_space="Shared")

            for b in range(BSZ):
                kt_t = KTP.tile([HD, PLEN], f16)
                nc.sync.dma_start(out=kt_t, in_=kT[b])
                v_t = VTP.tile([HD, NCH, HD + 1], f16)
                nc.sync.dma_start(out=v_t[:, :, 0:HD],
                                  in_=vv[b].rearrange("(c p) d -> p c d", p=HD))
                nc.vector.memset(v_t[:, :, HD:HD + 1], 1.0)

                qt_u = qT_sb[:, 0:G, b * QLEN:(b + 1) * QLEN]   # [128, 4, 16]
                o_ps = PSO.tile([TOK, HD + 1], f32)
                for kc in range(NCH):
                    s_ps = PSS.tile([HD, TOK], f32, tag="sc")
                    nc.tensor.matmul(s_ps, lhsT=kt_t[:, kc * HD:(kc + 1) * HD],
                                     rhs=qt_u, start=True, stop=True)
                    p_t = PTP.tile([HD, TOK], f16, tag="pt")
                    nc.scalar.activation(out=p_t, in_=s_ps, func=AF.Exp,
                                         scale=SCL, bias=mask_sb[:, kc:kc + 1])
                    nc.tensor.matmul(o_ps, lhsT=p_t, rhs=v_t[:, kc, :],
                                     start=(kc == 0), stop=False)
                # new-token chunk
                s_ps = PSS.tile([QLEN, TOK], f32, tag="sc")
                nc.tensor.matmul(s_ps, lhsT=qT_sb[:, G, b * QLEN:(b + 1) * QLEN],
                                 rhs=qt_u, start=True, stop=True)
                nc.vector.tensor_add(out=s_ps, in0=s_ps, in1=nmask_sb)
                p_t = PTP.tile([QLEN, TOK], f16, tag="pt")
                nc.scalar.activation(out=p_t, in_=s_ps, func=AF.Exp,
                                     scale=SCL, bias=shift_sb[:, 0:1])
                nc.tensor.matmul(o_ps, lhsT=p_t, rhs=v16[:, b, :],
                                 start=False, stop=True)

                # normalize by the ones-column accumulator, transpose, ship out
                rden = WK.tile([TOK, 1], f32, tag="rden")
                nc.vector.reciprocal(out=rden, in_=o_ps[:, HD:HD + 1])
                o_n = WK.tile([TOK, HD], f32, tag="on")
                nc.scalar.activation(out=o_n, in_=o_ps[:, 0:HD], func=AF.Copy,
                                     scale=rden[:, 0:1])
                ot_ps = PSM.tile([HD, TOK], f32, tag="misc")
                nc.tensor.transpose(ot_ps, o_n, ident32[0:TOK, 0:TOK])
                ot_sb = WK.tile([HD, TOK], f32, tag="ots")
                nc.vector.tensor_copy(out=ot_sb, in_=ot_ps)
                # contrib[g*128 + d, b*16 + s] = ot_sb[d, g*16 + s]
                nc.sync.dma_start(
                    out=bass.AP(tensor=contrib.tensor,
                                offset=contrib.offset + b * QLEN,
                                ap=[[TOK, HD], [HD * TOK, G], [1, QLEN]]),
                    in_=ot_sb.rearrange("p (g s) -> p g s", g=G))

            # ---------- AllGather ----------
            nc.gpsimd.collective_compute(
                "AllGather", ALU.bypass,
                replica_groups=[list(range(N_CORES))],
                ins=[contrib.opt()], outs=[gathered.opt()])

            # ---------- attn-out quantization + output projection ----------
            a_big = P1.tile([HD, HID // HD, TOK], f32)
            for qq in range(4):
                nc.sync.dma_start(
                    out=a_big[:, qq * 8:(qq + 1) * 8, :],
                    in_=gathered.rearrange("(c p) t -> p c t", p=HD)[:, qq * 8:(qq + 1) * 8, :])
            # per-token |max| via PE-transposed chunks (f32 DMA transpose n/a)
            amax = WK.tile([TOK, 1], f32, tag="amax")
            for kc in range(HID // HD):
                tp = PSM.tile([TOK, HD], f32, tag="misc")
                nc.tensor.transpose(tp, a_big[:, kc, :], ident32)
                pmax = WK.tile([TOK, 1], f32, tag="pmax")
                nc.vector.tensor_reduce(out=pmax, in_=tp, axis=AX.X, op=ALU.max,
                                        apply_absolute_value=True)
                if kc == 0:
                    nc.vector.tensor_copy(out=amax, in_=pmax)
                else:
                    nc.vector.tensor_max(out=amax, in0=amax, in1=pmax)
            s_at = P1.tile([TOK, 1], f32)
            nc.vector.tensor_scalar(out=s_at, in0=amax,
                                    scalar1=float(np.float32(1.0) / np.float32(127.0)),
                                    scalar2=1e-8, op0=ALU.mult, op1=ALU.max)
            rxs = WK.tile([TOK, 1], f32, tag="rxs")
            nc.vector.reciprocal(out=rxs, in_=s_at)
            rxs_ps = PSM.tile([1, TOK], f32, tag="misc")
            nc.tensor.transpose(rxs_ps, rxs, ident32[0:TOK, 0:TOK])
            rxs_row = WK.tile([1, TOK], f32, tag="rxsr")
            nc.vector.tensor_copy(out=rxs_row, in_=rxs_ps)
            rxs_dram = DR.tile([1, TOK], f32)
            nc.sync.dma_start(out=rxs_dram, in_=rxs_row)
            rxs_b = P1.tile([HD, TOK], f32)
            nc.sync.dma_start(out=rxs_b, in_=bass.AP(
                tensor=rxs_dram.tensor, offset=rxs_dram.offset,
                ap=[[0, HD]] + rxs_dram.ap[1:]))

            o_ps2 = PSP.tile([TOK, OW_ROWS], f32, tag="proj")
            t_i = P1.tile([HD, HID // HD, TOK], i32)
            q_at = P1.tile([HD, HID // HD, TOK], bf16)
            for kc in range(HID // HD):
                nc.vector.tensor_mul(out=t_i[:, kc, :], in0=a_big[:, kc, :],
                                     in1=rxs_b)
                nc.vector.tensor_copy(out=q_at[:, kc, :], in_=t_i[:, kc, :])
                nc.tensor.matmul(o_ps2, lhsT=q_at[:, kc, :], rhs=owT_sb[:, kc, :],
                                 start=(kc == 0), stop=(kc == HID // HD - 1))
            o_sb = P1.tile([TOK, OW_ROWS], f32)
            nc.scalar.activation(out=o_sb, in_=o_ps2, func=AF.Copy,
                                 scale=s_at[:, 0:1])
            nc.vector.tensor_mul(out=o_sb, in0=o_sb, in1=ows_b)
            nc.sync.dma_start(out=out_ap, in_=o_sb)

    nc.compile()
    return nc


def _quant_rows(w):
    s = np.maximum(np.max(np.abs(w), axis=1, keepdims=True)
                   / np.float32(127.0), np.float32(1e-8)).astype(np.float32)
    q = np.clip(np.round(w / s), -127.0, 127.0).astype(np.float32)
    return q, s[:, 0]


def kernel(x, past_k, past_v, qkv_w, o_w, q_len, num_full_kv_head,
           sink_size, recent_size):
    q_len = int(q_len); nf = int(num_full_kv_head)
    sink = int(sink_size); recent = int(recent_size)
    assert q_len == QLEN and nf == 4 and sink == 64 and recent == 1024, \
        "kernel compiled for q_len=16, nf=4, sink=64, recent=1024"
    x = np.asarray(x, np.float32)
    past_k = np.asarray(past_k, np.float32)
    past_v = np.asarray(past_v, np.float32)
    qkv_w = np.asarray(qkv_w, np.float32)
    o_w = np.asarray(o_w, np.float32)
    bf16 = ml_dtypes.bfloat16

    # ---- host prep
    xs = np.maximum(np.max(np.abs(x), axis=1, keepdims=True)
                    / np.float32(127.0), np.float32(1e-8)).astype(np.float32)
    xq = np.clip(np.round(x / xs), -127.0, 127.0).astype(np.float32)
    xqT = np.ascontiguousarray(xq.T).astype(bf16)

    wq, ws = _quant_rows(qkv_w)
    owq, ows_all = _quant_rows(o_w)

    # RoPE tables (f32 end-to-end, matching the jax reference ops)
    d_half = np.arange(0, HD, 2, dtype=np.float32) / np.float32(HD)
    inv_freq = (np.float32(1.0)
                / np.power(np.float32(10000.0), d_half)).astype(np.float32)
    pos = (PLEN + np.arange(QLEN)).astype(np.float32)
    ang = pos[:, None] * inv_freq[None, :]
    cos16 = np.cos(ang).astype(np.float32)
    sin16 = np.sin(ang).astype(np.float32)
    cosT = np.tile(cos16, (BSZ, 1))
    sinT = np.tile(sin16, (BSZ, 1))

    nm = np.full((QLEN, TOK), NEG, np.float32)
    r = np.arange(QLEN)[:, None]
    s = (np.arange(TOK) % QLEN)[None, :]
    nm[r <= s] = 0.0

    mv_full = np.full(PLEN, SHIFT, np.float32)
    sl = sink + recent                     # 1088 real streaming positions
    mv_str = np.concatenate([np.full(sl, SHIFT, np.float32),
                             np.full(PLEN - sl, NEG, np.float32)])

    in_maps = []
    for c in range(N_CORES):
        w_c = np.concatenate([
            wq[c * G * HD:(c + 1) * G * HD],
            wq[HID + c * HD:HID + (c + 1) * HD],
            wq[HID + NKV * HD + c * HD:HID + NKV * HD + (c + 1) * HD]], axis=0)
        ws_c = np.concatenate([
            ws[c * G * HD:(c + 1) * G * HD],
            ws[HID + c * HD:HID + (c + 1) * HD],
            ws[HID + NKV * HD + c * HD:HID + NKV * HD + (c + 1) * HD]])
        kT_c = np.zeros((BSZ, HD, PLEN), np.float16)
        vv_c = np.zeros((BSZ, PLEN, HD), np.float16)
        if c < nf:
            for b in range(BSZ):
                kT_c[b] = past_k[b, :, c, :].T.astype(np.float16)
                vv_c[b] = past_v[b, :, c, :].astype(np.float16)
            mv = mv_full
        else:
            for b in range(BSZ):
                kk = np.concatenate([past_k[b, :sink, c],
                                     past_k[b, PLEN - recent:, c]], axis=0)
                vvv = np.concatenate([past_v[b, :sink, c],
                                      past_v[b, PLEN - recent:, c]], axis=0)
                kT_c[b, :, :sl] = kk.T.astype(np.float16)
                vv_c[b, :sl] = vvv.astype(np.float16)
            mv = mv_str
        in_maps.append({
            "xqT": xqT, "xs": xs,
            "wqT": np.ascontiguousarray(w_c.T).astype(bf16),
            "wss": np.ascontiguousarray(ws_c[None, :]),
            "cosT": cosT, "sinT": sinT,
            "kT": kT_c, "vv": vv_c,
            "maskv": np.ascontiguousarray(mv.reshape(NCH, HD).T),
            "newmask": nm,
            "owT": np.ascontiguousarray(
                owq[c * OW_ROWS:(c + 1) * OW_ROWS].T).astype(bf16),
            "ows": np.ascontiguousarray(
                ows_all[None, c * OW_ROWS:(c + 1) * OW_ROWS]),
        })

    global _last_in_maps
    _last_in_maps = in_maps
    if "nc" not in _prog_cache:
        _prog_cache["nc"] = _build_program()
    nc = _prog_cache["nc"]

    res = bass_utils.run_bass_kernel_spmd(nc, in_maps,
                                          core_ids=list(range(N_CORES)))
    out = np.empty((TOK, HID), np.float32)
    for c in range(N_CORES):
        out[:, c * OW_ROWS:(c + 1) * OW_ROWS] = res.results[c]["out_slice"]
    return out



# revision 23
# speedup vs baseline: 1.3362x; 1.3362x over previous
"""Trainium2 Bass kernel for nn_LlamaAttention_31782757990403.

Sparse (full + streaming) Llama attention block with W8A8 fake-quant
projections, distributed over 8 NeuronCores.

Sharding (uniform SPMD - one NEFF, no branches):
  Core c owns kv head c (query heads 4c..4c+3).
    - QKV projection: output-channel shard (768 rows of qkv_w per core).
      Weights + activations are DMA'd as int8 (values are exact int8 from
      the fake-quant) and upconverted to bf16 on the DVE/Act engines,
      halving weight HBM traffic with zero accuracy loss.
    - Attention: 4 units = 4 batches of head c. KV is host-packed into
      33 position slots of 128 (32 past chunks + 1 tail slot); streaming
      heads carry sink+recent in slots 0..7 + tail, with the rest closed
      via per-(partition,group) additive bias columns (-1e9).
      Scores run in groups of 8 chunks per PSUM bank so one Exp
      activation covers [128, 512].
    - Attention outputs are AllGathered in f16 (feature-major) and
      requantized on device; o_w is row-sharded (512 out rows per core,
      also int8-DMA'd + upconverted); host concatenates the 8 slices.

Numerics:
  - int8 fake-quant values are exact in bf16; matmuls accumulate f32.
  - Attention in fp16 with a constant -4 exp shift (cancels in softmax).
  - attn-out quantization: round via f32->int32 convert (RNE).
"""

import numpy as np
import ml_dtypes

import concourse.bass as bass
import concourse.mybir as mybir
import concourse.tile as tile
from concourse import bacc, bass_utils
from concourse.masks import make_identity

dt = mybir.dt
AF = mybir.ActivationFunctionType
ALU = mybir.AluOpType
AX = mybir.AxisListType

NH, NKV, HD, HID = 32, 8, 128, 4096
BSZ, QLEN, PLEN = 4, 16, 4096
TOK = BSZ * QLEN                      # 64
G = NH // NKV                         # 4 query heads per kv head
N_CORES = 8
QKV_ROWS = G * HD + 2 * HD            # 768 rows of qkv_w per core
OW_ROWS = HID // N_CORES              # 512 o_w rows per core
NCH = PLEN // HD                      # 32 past-kv chunks of 128
NSLOT = NCH + 1                       # 33: 32 chunks + tail slot
NGRP = 4                              # chunk groups of 8 per unit
SCL = float(1.0 / np.sqrt(np.float32(HD)))   # 1/sqrt(128)
SHIFT = -4.0                          # exp stability shift (cancels)
NEG = -1.0e9

_prog_cache = {}


def _build_program():
    nc = bacc.Bacc("TRN2", target_bir_lowering=False, debug=False,
                   num_devices=N_CORES)
    f32, f16, bf16, i32, i8 = (dt.float32, dt.float16, dt.bfloat16,
                               dt.int32, dt.int8)

    def inp(name, shape, d):
        return nc.dram_tensor(name, shape, d, kind="ExternalInput").ap()

    MISC_COLS = 1 + 64 + 64 + TOK + QKV_ROWS + OW_ROWS   # 1473
    KV_COLS = NSLOT * HD + NSLOT * (HD + 1)              # 8481
    xqT8 = inp("xqT8", [HD, HID // HD, TOK], i8)
    wqT8 = inp("wqT8", [HD, HID // HD, QKV_ROWS], i8)
    misc = inp("misc", [TOK, MISC_COLS], f32)
    kvpack = inp("kvpack", [BSZ, HD, KV_COLS], f16)
    btab = inp("btab", [HD, BSZ * 5], f32)
    owT8 = inp("owT8", [HD, HID // HD, OW_ROWS], i8)
    out_ap = nc.dram_tensor("out_slice", [TOK, OW_ROWS], f32,
                            kind="ExternalOutput").ap()

    with tile.TileContext(nc, num_cores=N_CORES) as tc:
        with (
            tc.tile_pool(name="persist", bufs=1) as P1,
            tc.tile_pool(name="pt", bufs=3) as PTP,
            tc.tile_pool(name="work", bufs=2) as WK,
            tc.tile_pool(name="kvp", bufs=2) as KVP,
            tc.tile_pool(name="ps_sc", bufs=2, space="PSUM") as PSS,
            tc.tile_pool(name="ps_o", bufs=2, space="PSUM") as PSO,
            tc.tile_pool(name="ps_proj", bufs=2, space="PSUM") as PSP,
            tc.tile_pool(name="ps_m", bufs=2, space="PSUM") as PSM,
            tc.tile_pool(name="dram", bufs=1, space="DRAM") as DR,
        ):
            # ---------- DMA loads (SP queue; few, large, contiguous) -------
            # wq8 arrives as two halves so upconverts + matmuls start at the
            # half-way mark; ow8 is dispatched from the Act HWDGE queue so the
            # scheduler cannot hoist its transfer ahead of the KV loads.
            xq8_sb = P1.tile([HD, HID // HD, TOK], i8)
            nc.sync.dma_start(out=xq8_sb, in_=xqT8)
            wq8_sb = P1.tile([HD, HID // HD, QKV_ROWS], i8)
            HALF = HID // HD // 2
            nc.sync.dma_start(out=wq8_sb[:, :HALF, :], in_=wqT8[:, :HALF, :])
            nc.sync.dma_start(out=wq8_sb[:, HALF:, :], in_=wqT8[:, HALF:, :])
            misc_sb = P1.tile([TOK, MISC_COLS], f32)
            nc.scalar.dma_start(out=misc_sb, in_=misc)
            btab_sb = P1.tile([HD, BSZ * 5], f32)
            nc.scalar.dma_start(out=btab_sb, in_=btab)
            kv = []
            for u in range(BSZ):
                kv_u = KVP.tile([HD, KV_COLS], f16, tag="kv")
                nc.sync.dma_start(out=kv_u, in_=kvpack[u])
                probe = WK.tile([HD, 1], f16, tag="kvprobe")
                nc.gpsimd.tensor_copy(out=probe, in_=kv_u[:, 0:1])
                kv.append(kv_u)
            # o_w int8 stages into the wq8 tile (dead after the upconverts)
            ow8_sb = wq8_sb[:, :, 0:OW_ROWS]

            xs_sb = misc_sb[:, 0:1]
            cos_sb = misc_sb[:, 1:65]
            sin_sb = misc_sb[:, 65:129]
            nmask_sb = misc_sb[0:QLEN, 129:129 + TOK]
            wss_b = misc_sb[:, 193:193 + QKV_ROWS]
            ows_b = misc_sb[:, 961:961 + OW_ROWS]

            VOFF = NSLOT * HD                 # v columns start in kv_u

            def kslot(u, s):
                return kv[u][:, s * HD:(s + 1) * HD]

            def vslot(u, s):
                return kv[u][:, VOFF + s * (HD + 1):VOFF + (s + 1) * (HD + 1)]

            ident16 = P1.tile([HD, HD], f16)
            make_identity(nc, ident16)
            ident32 = P1.tile([HD, HD], f32)
            make_identity(nc, ident32)
            shift_sb = P1.tile([QLEN, 1], f32)
            nc.vector.memset(shift_sb, SHIFT)

            # ---------- int8 -> bf16 upconverts (DVE + Act split) ----------
            # chunked in k-order pieces so the QKV matmuls pipeline behind
            # the converts instead of waiting for the whole weight tensor
            xq_sb = P1.tile([HD, HID // HD, TOK], bf16)
            nc.vector.tensor_copy(out=xq_sb, in_=xq8_sb)
            wq_sb = P1.tile([HD, HID // HD, QKV_ROWS], bf16)
            for p0 in range(0, HID // HD, 8):
                nc.vector.tensor_copy(out=wq_sb[:, p0:p0 + 4, :],
                                      in_=wq8_sb[:, p0:p0 + 4, :])
                nc.scalar.activation(out=wq_sb[:, p0 + 4:p0 + 8, :],
                                     in_=wq8_sb[:, p0 + 4:p0 + 8, :], func=AF.Copy)
            nc.scalar.dma_start(out=ow8_sb, in_=owT8)

            # ---------- QKV projection ----------
            qkv_sb = P1.tile([TOK, QKV_ROWS], f32)
            for nb in range(2):
                ncols = QKV_ROWS // 2    # 384
                ps = PSP.tile([TOK, ncols], f32, tag="proj",
                              padded_shape=[TOK, 512])
                for kc in range(HID // HD):
                    nc.tensor.matmul(
                        ps, lhsT=xq_sb[:, kc, :],
                        rhs=wq_sb[:, kc, nb * ncols:(nb + 1) * ncols],
                        start=(kc == 0), stop=(kc == HID // HD - 1))
                nc.scalar.activation(out=qkv_sb[:, nb * ncols:(nb + 1) * ncols],
                                     in_=ps, func=AF.Copy, scale=xs_sb[:, 0:1])
            nc.vector.tensor_mul(out=qkv_sb, in0=qkv_sb, in1=wss_b)

            # ---------- RoPE on q (4 heads) and k ----------
            roped = P1.tile([TOK, (G + 1) * HD], f32)
            for seg in range(G + 1):
                b0 = seg * HD
                x1 = qkv_sb[:, b0:b0 + 64]
                x2 = qkv_sb[:, b0 + 64:b0 + HD]
                t1 = WK.tile([TOK, 64], f32, tag="rope1")
                t2 = WK.tile([TOK, 64], f32, tag="rope2")
                nc.vector.tensor_mul(out=t1, in0=x1, in1=cos_sb)
                nc.vector.tensor_mul(out=t2, in0=x2, in1=sin_sb)
                nc.vector.tensor_sub(out=roped[:, b0:b0 + 64], in0=t1, in1=t2)
                nc.vector.tensor_mul(out=t1, in0=x1, in1=sin_sb)
                nc.vector.tensor_mul(out=t2, in0=x2, in1=cos_sb)
                nc.vector.tensor_add(out=roped[:, b0 + 64:b0 + HD], in0=t1, in1=t2)

            qk16 = P1.tile([TOK, (G + 1) * HD], f16)
            nc.vector.tensor_copy(out=qk16, in_=roped)
            v16f = P1.tile([TOK, HD], f16)
            nc.vector.tensor_copy(out=v16f, in_=qkv_sb[:, (G + 2) * HD - HD:])
            # per-batch v tiles at base partition 0: [16 tok, 4 batch, 129]
            v16 = P1.tile([QLEN, BSZ, HD + 1], f16)
            nc.vector.memset(v16[:, :, HD:HD + 1], 1.0)
            vT_ps = PSM.tile([HD, TOK], f16, tag="misc")
            nc.tensor.transpose(vT_ps, v16f, ident16[0:TOK, 0:TOK])
            vTs = P1.tile([HD, TOK], f16)
            nc.vector.tensor_copy(out=vTs, in_=vT_ps)
            for b in range(BSZ):
                vb_ps = PSM.tile([QLEN, HD], f16, tag="misc")
                nc.tensor.transpose(vb_ps, vTs[:, b * QLEN:(b + 1) * QLEN],
                                    ident16)
                nc.vector.tensor_copy(out=v16[:, b, 0:HD], in_=vb_ps)

            # transpose q heads + k: [64, 128] -> [128, 64]
            qT_sb = P1.tile([HD, G + 1, TOK], f16)
            for seg in range(G + 1):
                pst = PSM.tile([HD, TOK], f16, tag="misc")
                nc.tensor.transpose(pst, qk16[:, seg * HD:(seg + 1) * HD],
                                    ident16[0:TOK, 0:TOK])
                nc.vector.tensor_copy(out=qT_sb[:, seg, :], in_=pst)

            # ---------- attention units (4 batches of this core's kv head) --
            contrib = DR.tile([HD, G * TOK], f16)
            gathered = DR.tile([N_CORES * HD, G * TOK], f16, addr_space="Shared")

            for u in range(BSZ):
                q_u = qT_sb[:, 0:G, u * QLEN:(u + 1) * QLEN]   # [128, 4, 16]
                o_ps = PSO.tile([TOK, HD + 1], f32, tag="ops")
                for g in range(NGRP):
                    s_ps = PSS.tile([HD, 8 * TOK], f32, tag="sc")
                    for j in range(8):
                        nc.tensor.matmul(s_ps[:, j * TOK:(j + 1) * TOK],
                                         lhsT=kslot(u, g * 8 + j),
                                         rhs=q_u, start=True, stop=True)
                    p_t = PTP.tile([HD, 8 * TOK], f16, tag="pt")
                    nc.scalar.activation(out=p_t, in_=s_ps, func=AF.Exp,
                                         scale=SCL,
                                         bias=btab_sb[:, u * 5 + g:u * 5 + g + 1])
                    for j in range(8):
                        nc.tensor.matmul(o_ps, lhsT=p_t[:, j * TOK:(j + 1) * TOK],
                                         rhs=vslot(u, g * 8 + j),
                                         start=(g == 0 and j == 0), stop=False)
                # tail slot (streaming sink/recent remainder)
                s2 = PSM.tile([HD, TOK], f32, tag="misc")
                nc.tensor.matmul(s2, lhsT=kslot(u, NCH), rhs=q_u,
                                 start=True, stop=True)
                p2 = PTP.tile([HD, TOK], f16, tag="pt2")
                nc.scalar.activation(out=p2, in_=s2, func=AF.Exp, scale=SCL,
                                     bias=btab_sb[:, u * 5 + 4:u * 5 + 5])
                nc.tensor.matmul(o_ps, lhsT=p2, rhs=vslot(u, NCH),
                                 start=False, stop=False)
                # new-token chunk (causal)
                s3 = PSM.tile([QLEN, TOK], f32, tag="misc")
                nc.tensor.matmul(s3, lhsT=qT_sb[:, G, u * QLEN:(u + 1) * QLEN],
                                 rhs=q_u, start=True, stop=True)
                nc.vector.tensor_add(out=s3, in0=s3, in1=nmask_sb)
                p3 = PTP.tile([QLEN, TOK], f16, tag="pt3")
                nc.scalar.activation(out=p3, in_=s3, func=AF.Exp,
                                     scale=SCL, bias=shift_sb[:, 0:1])
                nc.tensor.matmul(o_ps, lhsT=p3, rhs=v16[:, u, :],
                                 start=False, stop=True)

                # normalize by the ones-column accumulator, transpose, ship out
                rden = WK.tile([TOK, 1], f32, tag="rden")
                nc.vector.reciprocal(out=rden, in_=o_ps[:, HD:HD + 1])
                o_n = WK.tile([TOK, HD], f32, tag="on")
                nc.scalar.activation(out=o_n, in_=o_ps[:, 0:HD], func=AF.Copy,
                                     scale=rden[:, 0:1])
                ot_ps = PSM.tile([HD, TOK], f32, tag="misc")
                nc.tensor.transpose(ot_ps, o_n, ident32[0:TOK, 0:TOK])
                ot_sb = WK.tile([HD, TOK], f16, tag="ots")
                nc.vector.tensor_copy(out=ot_sb, in_=ot_ps)
                # contrib[d, qh*64 + u*16 + s] = ot_sb[d, qh*16 + s]
                nc.gpsimd.dma_start(
                    out=bass.AP(tensor=contrib.tensor,
                                offset=contrib.offset + u * QLEN,
                                ap=[[G * TOK, HD], [TOK, G], [1, QLEN]]),
                    in_=ot_sb.rearrange("p (g s) -> p g s", g=G))

            # ---------- AllGather (f16, feature-major) ---------------------
            nc.gpsimd.collective_compute(
                "AllGather", ALU.bypass,
                replica_groups=[list(range(N_CORES))],
                ins=[contrib.opt()], outs=[gathered.opt()])

            # ---------- o_w upconvert (Pool engine; textually after the
            # collective dispatch so it fills the idle collective window).
            # ow_sb reuses wq_sb's space (dead after the QKV projection).
            ow_sb = wq_sb[:, :, 0:OW_ROWS]
            for p0 in range(0, HID // HD, 4):
                nc.gpsimd.tensor_copy(out=ow_sb[:, p0:p0 + 4, :],
                                      in_=ow8_sb[:, p0:p0 + 4, :])

            # ---------- attn-out requantization + output projection --------
            a_sb = P1.tile([HD, N_CORES, G * TOK], f16)
            nc.sync.dma_start(out=a_sb, in_=gathered.rearrange(
                "(c p) x -> p c x", p=HD))
            # per-(d, token) |max| over the 32 head-chunks: strided X reduce
            r1 = WK.tile([HD, TOK], f32, tag="r1")
            nc.vector.tensor_reduce(
                out=r1,
                in_=bass.AP(tensor=a_sb.tensor, offset=a_sb.offset,
                            ap=[a_sb.ap[0], [1, TOK], [TOK, NH]]),
                axis=AX.X, op=ALU.max, apply_absolute_value=True)
            r1t_ps = PSM.tile([TOK, HD], f32, tag="misc")
            nc.tensor.transpose(r1t_ps, r1, ident32)
            r1t = WK.tile([TOK, HD], f32, tag="r1t")
            nc.vector.tensor_copy(out=r1t, in_=r1t_ps)
            amax = WK.tile([TOK, 1], f32, tag="amax")
            nc.vector.tensor_reduce(out=amax, in_=r1t, axis=AX.X, op=ALU.max)
            s_at = P1.tile([TOK, 1], f32)
            nc.vector.tensor_scalar(out=s_at, in0=amax,
                                    scalar1=float(np.float32(1.0) / np.float32(127.0)),
                                    scalar2=1e-8, op0=ALU.mult, op1=ALU.max)
            rxs = WK.tile([TOK, 1], f32, tag="rxs")
            nc.vector.reciprocal(out=rxs, in_=s_at)
            rxs_ps = PSM.tile([1, TOK], f32, tag="misc")
            nc.tensor.transpose(rxs_ps, rxs, ident32[0:TOK, 0:TOK])
            # replicate across qh on the row, then PE-broadcast to 128
            # partitions via a rank-1 matmul with a ones column
            rxs_rep = WK.tile([1, G * TOK], f32, tag="rxsr")
            for qh in range(G):
                nc.vector.tensor_copy(out=rxs_rep[:, qh * TOK:(qh + 1) * TOK],
                                      in_=rxs_ps)
            ones1 = P1.tile([1, HD], f32)
            nc.vector.memset(ones1, 1.0)
            rxs_bps = PSM.tile([HD, G * TOK], f32, tag="misc")
            nc.tensor.matmul(rxs_bps, lhsT=ones1, rhs=rxs_rep,
                             start=True, stop=True)
            rxs_b4 = P1.tile([HD, G, TOK], f32)
            nc.vector.tensor_copy(out=rxs_b4, in_=rxs_bps)

            o_ps2 = PSP.tile([TOK, OW_ROWS], f32, tag="proj",
                              padded_shape=[TOK, 512])
            for cb in range(0, N_CORES, 2):
                t_i = WK.tile([HD, 2, G * TOK], i32, tag="ti")
                nc.vector.tensor_mul(out=t_i, in0=a_sb[:, cb:cb + 2, :],
                                     in1=bass.AP(
                                         tensor=rxs_b4.tensor,
                                         offset=rxs_b4.offset,
                                         ap=[rxs_b4.ap[0], [0, 2], [1, G * TOK]]))
                q_at = WK.tile([HD, 2, G * TOK], bf16, tag="qat")
                nc.scalar.activation(out=q_at, in_=t_i, func=AF.Copy)
                for half in range(2):
                    for qh in range(G):
                        kc = (cb + half) * G + qh
                        nc.tensor.matmul(o_ps2,
                                         lhsT=q_at[:, half, qh * TOK:(qh + 1) * TOK],
                                         rhs=ow_sb[:, kc, :],
                                         start=(kc == 0), stop=(kc == NH - 1))
            o_sb = P1.tile([TOK, OW_ROWS], f32)
            nc.scalar.activation(out=o_sb, in_=o_ps2, func=AF.Copy,
                                 scale=s_at[:, 0:1])
            nc.vector.tensor_mul(out=o_sb, in0=o_sb, in1=ows_b)
            nc.sync.dma_start(out=out_ap, in_=o_sb)

    nc.compile()
    return nc


def _quant_rows(w):
    s = np.maximum(np.max(np.abs(w), axis=1, keepdims=True)
                   / np.float32(127.0), np.float32(1e-8)).astype(np.float32)
    q = np.clip(np.round(w / s), -127.0, 127.0).astype(np.float32)
    return q, s[:, 0]


def _pack_w8(wq_rows):
    """[rows, 4096] int-valued -> [128, 32, rows] int8 (p, c, f) layout."""
    r = wq_rows.shape[0]
    return np.ascontiguousarray(
        wq_rows.T.reshape(HID // HD, HD, r).transpose(1, 0, 2)).astype(np.int8)


def kernel(x, past_k, past_v, qkv_w, o_w, q_len, num_full_kv_head,
           sink_size, recent_size):
    q_len = int(q_len); nf = int(num_full_kv_head)
    sink = int(sink_size); recent = int(recent_size)
    assert q_len == QLEN and nf == 4 and sink == 64 and recent == 1024, \
        "kernel compiled for q_len=16, nf=4, sink=64, recent=1024"
    x = np.asarray(x, np.float32)
    past_k = np.asarray(past_k, np.float32)
    past_v = np.asarray(past_v, np.float32)
    qkv_w = np.asarray(qkv_w, np.float32)
    o_w = np.asarray(o_w, np.float32)

    # ---- host prep
    xs = np.maximum(np.max(np.abs(x), axis=1, keepdims=True)
                    / np.float32(127.0), np.float32(1e-8)).astype(np.float32)
    xq = np.clip(np.round(x / xs), -127.0, 127.0).astype(np.float32)
    xqT8 = _pack_w8(xq)                      # [128, 32, 64] int8

    wq, ws = _quant_rows(qkv_w)
    owq, ows_all = _quant_rows(o_w)

    # RoPE tables (f32 end-to-end, matching the jax reference ops)
    d_half = np.arange(0, HD, 2, dtype=np.float32) / np.float32(HD)
    inv_freq = (np.float32(1.0)
                / np.power(np.float32(10000.0), d_half)).astype(np.float32)
    pos = (PLEN + np.arange(QLEN)).astype(np.float32)
    ang = pos[:, None] * inv_freq[None, :]
    cos16 = np.cos(ang).astype(np.float32)
    sin16 = np.sin(ang).astype(np.float32)
    cosT = np.tile(cos16, (BSZ, 1))
    sinT = np.tile(sin16, (BSZ, 1))

    nm = np.full((QLEN, TOK), NEG, np.float32)
    r = np.arange(QLEN)[:, None]
    s = (np.arange(TOK) % QLEN)[None, :]
    nm[r <= s] = 0.0
    nm64 = np.zeros((TOK, TOK), np.float32)
    nm64[:QLEN] = nm

    sl = sink + recent                     # 1088 real streaming positions
    in_maps = []
    for c in range(N_CORES):
        w_c = np.concatenate([
            wq[c * G * HD:(c + 1) * G * HD],
            wq[HID + c * HD:HID + (c + 1) * HD],
            wq[HID + NKV * HD + c * HD:HID + NKV * HD + (c + 1) * HD]], axis=0)
        ws_c = np.concatenate([
            ws[c * G * HD:(c + 1) * G * HD],
            ws[HID + c * HD:HID + (c + 1) * HD],
            ws[HID + NKV * HD + c * HD:HID + NKV * HD + (c + 1) * HD]])

        kp = np.zeros((BSZ, NSLOT * HD, HD), np.float16)
        vp = np.zeros((BSZ, NSLOT * HD, HD + 1), np.float16)
        vp[:, :, HD] = 1.0
        bt = np.full((BSZ, 5, HD), NEG, np.float32)
        if c < nf:
            for b in range(BSZ):
                kp[b, :PLEN] = past_k[b, :, c, :].astype(np.float16)
                vp[b, :PLEN, :HD] = past_v[b, :, c, :].astype(np.float16)
            bt[:, :4, :] = SHIFT                 # 4 open groups, tail closed
        else:
            for b in range(BSZ):
                kk = np.concatenate([past_k[b, :sink, c],
                                     past_k[b, PLEN - recent:, c]], axis=0)
                vv = np.concatenate([past_v[b, :sink, c],
                                     past_v[b, PLEN - recent:, c]], axis=0)
                kp[b, :1024] = kk[:1024].astype(np.float16)
                vp[b, :1024, :HD] = vv[:1024].astype(np.float16)
                kp[b, NCH * HD:NCH * HD + 64] = kk[1024:].astype(np.float16)
                vp[b, NCH * HD:NCH * HD + 64, :HD] = vv[1024:].astype(np.float16)
            bt[:, 0, :] = SHIFT                  # group 0 open
            bt[:, 4, :64] = SHIFT                # tail: first 64 positions open
        # k device layout: partitions = head-dim d, columns = global position
        # (slot-major); v: partitions = position-within-slot, columns (slot, d)
        kpack = np.ascontiguousarray(kp.transpose(0, 2, 1))
        vpack = (vp.reshape(BSZ, NSLOT, HD, HD + 1).transpose(0, 2, 1, 3)
                 .reshape(BSZ, HD, NSLOT * (HD + 1)))
        kvpack = np.ascontiguousarray(
            np.concatenate([kpack, vpack], axis=2))
        btab = np.ascontiguousarray(
            bt.transpose(2, 0, 1).reshape(HD, BSZ * 5))
        misc = np.ascontiguousarray(np.concatenate([
            xs, cosT, sinT, nm64,
            np.broadcast_to(ws_c[None, :], (TOK, QKV_ROWS)),
            np.broadcast_to(ows_all[None, c * OW_ROWS:(c + 1) * OW_ROWS],
                            (TOK, OW_ROWS)),
        ], axis=1).astype(np.float32))

        in_maps.append({
            "xqT8": xqT8,
            "wqT8": _pack_w8(w_c),
            "misc": misc,
            "kvpack": kvpack,
            "btab": btab,
            "owT8": _pack_w8(owq[c * OW_ROWS:(c + 1) * OW_ROWS]),
        })

    global _last_in_maps
    _last_in_maps = in_maps
    if "nc" not in _prog_cache:
        _prog_cache["nc"] = _build_program()
    nc = _prog_cache["nc"]

    res = bass_utils.run_bass_kernel_spmd(nc, in_maps,
                                          core_ids=list(range(N_CORES)))
    out = np.empty((TOK, HID), np.float32)
    for c in range(N_CORES):
        out[:, c * OW_ROWS:(c + 1) * OW_ROWS] = res.results[c]["out_slice"]
    return out
